# revision 40
# baseline (speedup 1.0000x reference)
"""Fused multi-head attention + output projection for Trainium2 (Bass/Tile).

Problem: B=4, N=2048, C=768, H=12 heads x D=64.
  qkv = x @ W_qkv + b_qkv ; q,k,v per head ; attn = softmax(q k^T / sqrt(D))
  attn_out = (attn @ v) merged ; out = attn_out @ W_proj + b_proj
  returns (out, attn_out)

Sharding over 8 NeuronCores: core c = (b, g) with b = batch (4), g = head
group (2 groups of 6 heads).  Data-parallel over batch, tensor-parallel over
heads: W_qkv columns / W_proj rows are split per group; the N x N attention
matrix stays core-local.  Host only slices inputs and, on gather, transposes
the (feature-major) outputs and sums the two W_proj partial products per
batch.

Per-core device algorithm (all layouts feature-major "T" = [features, n]):
  xT = transpose(x_b)                       (PE transposes via identity)
  qkT[f, n] = W_qk^T x (+bias, q pre-scaled on host)      fp32r matmuls
  v[n, f] (+bias via ones-row matmul), stored bf16 augmented with a ones
      column per head -> PV matmul also yields softmax row-sums.
  Per head: S^T[k, q] = kT^T qT (no max subtraction needed: |S| <= ~6),
      P^T = exp(S^T) on ScalarE straight out of PSUM (bf16),
      ctx^T[d, q] (+rowsum row) = [v|1]^T @ P^T, normalize by 1/rowsum.
  out^T = W_proj^T ctx^T (+b_proj on group-0 cores only, via zeroed input).

All phases share one 6-bank PSUM "ring" of [128,512] units (plus 2 banks of
PV accumulators), so no PSUM pool boundary serializes phase transitions.
"""

import os
import numpy as np
import ml_dtypes
from contextlib import ExitStack

import concourse.bass as bass
import concourse.tile as tile
import concourse.mybir as mybir
from concourse import bacc
import concourse.bass_utils as _bass_utils
from concourse.bass_utils import run_bass_kernel_spmd

# walrus is invoked with --enable-ldw-opt=false by default, which forces a
# serial LDWEIGHTS before every MATMUL (~250us of PE time for this kernel).
_orig_run_command = _bass_utils.run_command


def _run_command_ldw(argv, **kw):
    argv = ["--enable-ldw-opt=true" if a == "--enable-ldw-opt=false" else a
            for a in argv]
    return _orig_run_command(argv, **kw)


# NOTE: tried --enable-ldw-opt=true: walrus rejects it for fp32/fp32r
# weights ("InstLdweights is not compatible with LDW optimization").
ENABLE_LDW_OPT = bool(os.environ.get("K_LDW_OPT"))
if ENABLE_LDW_OPT and _bass_utils.run_command is _orig_run_command:
    _bass_utils.run_command = _run_command_ldw

N_CORES = 8
B, N, C = 4, 2048, 768
H, D = 12, 64
G = 2                # head groups (tensor-parallel)
HL = H // G          # heads per core
CL = HL * D          # local feature width (384)
SCALE = D ** -0.5
NT = N // 128        # 16 row tiles
CC = C // 128        # 6 contraction chunks
QC = N // 512        # 4 q chunks of 512
KT = N // 128        # 16 k tiles
FQK = 2 * CL // 128  # 6 feature tiles for q|k

F32 = mybir.dt.float32
F32R = mybir.dt.float32r
BF16 = mybir.dt.bfloat16
I16 = mybir.dt.int16

# Schraudolph fast-exp in bf16 bit space: bf16_bits(exp(x)) ~ x*FE_A + FE_B.
# FE_B calibrated on HW (trunc semantics) for zero-mean relative error.
FE_A = 2.0 ** 7 / float(np.log(2.0))
FE_B = 16256.5 - 7.88
AF = mybir.ActivationFunctionType
ALU = mybir.AluOpType

_CACHE = {}


class Ring:
    """Rotating [128, 512] PSUM units across persistent 2-bank slot
    tiles. Separate tiles keep the tile-level dependency tracking
    per-slot: the ST matmul reusing slot s waits only on that slot's
    exp reader n_slots kts back, not on every in-flight ring access."""

    def __init__(self, slots):
        self.slots = slots          # tiles of [128, 1024]
        self.n = 2 * len(slots)
        self.pos = 0

    def unit(self, width=512):
        p = self.pos % self.n
        self.pos += 1
        return self.slots[p // 2][:, (p % 2) * 512:(p % 2) * 512 + width], p

    def slot_unit(self):
        p = self.pos % self.n
        assert p % 2 == 0
        self.pos += 2
        return self.slots[p // 2][:, :]

    def slot(self, r):
        return self.slots[r]


def _build_nc(repeat=1):
    nc = bacc.Bacc("TRN2", target_bir_lowering=False, debug=False,
                   num_devices=N_CORES)
    xt_d = nc.dram_tensor("xT", [C, N], BF16, kind="ExternalInput").ap()
    wqk_d = nc.dram_tensor("w_qk", [128, FQK * CC * 128], BF16,
                           kind="ExternalInput").ap()
    wv_d = nc.dram_tensor("w_v", [128, CC * CL], BF16, kind="ExternalInput").ap()
    wp_d = nc.dram_tensor("w_p", [128, 3 * C], BF16, kind="ExternalInput").ap()
    bqk_d = nc.dram_tensor("b_qk", [128, FQK], F32, kind="ExternalInput").ap()
    bv_d = nc.dram_tensor("b_v", [1, CL], F32, kind="ExternalInput").ap()
    bp_d = nc.dram_tensor("b_p", [128, C // 128], F32, kind="ExternalInput").ap()
    aot_d = nc.dram_tensor("attn_out_t", [CL, N], BF16, kind="ExternalOutput").ap()
    out_d = nc.dram_tensor("out_t", [C, N], BF16, kind="ExternalOutput").ap()
    DEBUG = bool(os.environ.get("K_DEBUG"))
    if DEBUG:
        dbg_rs = nc.dram_tensor("dbg_rs", [1, 512], F32, kind="ExternalOutput").ap()
        dbg_rA = nc.dram_tensor("dbg_rA", [1, 512], F32, kind="ExternalOutput").ap()
        dbg_bc = nc.dram_tensor("dbg_bc", [64, 512], F32, kind="ExternalOutput").ap()

    with tile.TileContext(nc) as tc:
      for _rep in range(repeat):
        with ExitStack() as top:
            const_pool = top.enter_context(tc.tile_pool(name="const", bufs=1))
            bias_pool = top.enter_context(tc.tile_pool(name="bias", bufs=3))
            qkT_pool = top.enter_context(tc.tile_pool(name="qkT", bufs=FQK))
            vaug_pool = top.enter_context(tc.tile_pool(name="vaug", bufs=NT))
            wp_pool = top.enter_context(tc.tile_pool(name="wp", bufs=1))
            ring_pool = top.enter_context(
                tc.tile_pool(name="ring", bufs=1, space="PSUM"))

            # 2 ring slots (4 banks) + 4 ctx accumulator banks = all 8
            # PSUM banks: the ctx double-buffering lets the PV backlog
            # carry across head-pair groups without draining the ring.
            ring = Ring([ring_pool.tile([128, 1024], F32, tag=f"ring{i}",
                                        name=f"ringt{i}") for i in range(2)])

            # x arrives pre-transposed from the host: plain parallel DMA
            # loads instead of the serialized transpose-xbar path.
            xT_pool_o = top.enter_context(tc.tile_pool(name="xT", bufs=CC))
            xT = [xT_pool_o.tile([128, N], BF16, tag="xT", name=f"xTt{i}")
                  for i in range(CC)]

            b_qk = bias_pool.tile([128, FQK], F32, tag="bqk")
            b_p = bias_pool.tile([128, C // 128], F32, tag="bp")
            b_v = bias_pool.tile([1, CL], F32, tag="bv")

            qkT = [qkT_pool.tile([128, N], BF16, tag="qkT", name=f"qkT{i}")
                   for i in range(FQK)]
            # per-head pitch 65: [v(64) | ones(1)] — the narrow stationary
            # keeps the PV ldweights at 65 columns (~54ns) instead of 128.
            vaug = [vaug_pool.tile([128, HL * 65], BF16, tag="vaug",
                                   name=f"vaug{i}") for i in range(NT)]
            # ------------- Phases A (xT), B (qkT), C (v) -------------
            with ExitStack() as s1:
                wv_pool = s1.enter_context(tc.tile_pool(name="wv", bufs=1))
                wv_all = wv_pool.tile([128, CC * CL], BF16, tag="wv",
                                      name="wv_all")
                wv = [wv_all[:, cc * CL:(cc + 1) * CL] for cc in range(CC)]

                wqk_pool = s1.enter_context(
                    tc.tile_pool(name="wqk", bufs=FQK))
                # ft-major weight slabs: one 0.2MB DMA unblocks a whole
                # B ft-group instead of needing all six row chunks.
                wqkF = [wqk_pool.tile([128, CC * 128], BF16, tag="wqk",
                                      name=f"wqkF{ft}") for ft in range(FQK)]
                cps_pool = s1.enter_context(
                    tc.tile_pool(name="cps", bufs=1, space="PSUM"))
                cunits = [cps_pool.tile([128, CL], F32, tag=f"cps{i}",
                                        name=f"cps{i}") for i in range(4)]

                def _dma_wqkF(ft):
                    nc.sync.dma_start(wqkF[ft][:],
                                      wqk_d[:, ft * 768:(ft + 1) * 768])

                # DMA issue order = need order. Weights ride the SP HWDGE
                # queue, xT the Act queue, so issue serialization (~0.7us
                # per dma_start) runs 2-wide. xT[0] is split so the very
                # first matmul starts after ~0.3 MB of traffic.
                _dma_wqkF(0)
                nc.scalar.dma_start(xT[0][:, 0:512], xt_d[0:128, 0:512])
                nc.sync.dma_start(wv_all[:], wv_d[:])
                nc.scalar.dma_start(xT[0][:, 512:N], xt_d[0:128, 512:N])
                for cc in range(1, CC):
                    nc.scalar.dma_start(xT[cc][:],
                                        xt_d[cc * 128:(cc + 1) * 128, :])
                nc.sync.dma_start(b_qk[:], bqk_d[:])
                nc.sync.dma_start(b_v[:], bv_d[:])
                for ft in range(1, FQK):
                    _dma_wqkF(ft)
                nc.sync.dma_start(b_p[:], bp_d[:])
                wp_all = wp_pool.tile([128, 3 * C], BF16, tag="wp",
                                      name="wp_all")
                nc.scalar.dma_start(wp_all[:], wp_d[:])
                wp = [wp_all[:, i * C:(i + 1) * C] for i in range(3)]

                # b_v broadcast once: the C evac folds the v-bias via
                # tensor_tensor, saving a 1-row matmul per nt tile.
                bvb = bias_pool.tile([128, CL], F32, tag="bvb")
                nc.gpsimd.partition_broadcast(bvb[:], b_v[:])

                def _evac_b(ft, qc, unit):
                    if qc % 2 == 0:
                        nc.vector.tensor_scalar_add(
                            qkT[ft][:, qc * 512:(qc + 1) * 512],
                            unit[:], b_qk[:, ft:ft + 1])
                    else:
                        nc.scalar.activation(
                            qkT[ft][:, qc * 512:(qc + 1) * 512], unit[:],
                            AF.Identity, bias=b_qk[:, ft:ft + 1])

                def _evac_c(nt, unit):
                    va3 = vaug[nt][:].rearrange("p (h e) -> p h e", e=65)
                    nc.vector.tensor_tensor(
                        va3[:, :, 0:64],
                        unit[:].rearrange("p (h e) -> p h e", e=64),
                        bvb[:].rearrange("p (h e) -> p h e", e=64),
                        op=ALU.add)
                    nc.vector.memset(va3[:, :, 64:65], 1.0)

                def _b_group(ft, c_sweep=None):
                    # B ft-group, cc-outer: the 4 q-chunk accumulators
                    # fill in DMA-arrival order so the PE streams each xT
                    # chunk as it lands; evacs ride inline with the last
                    # cc pass so the next group's units free up early.
                    # c_sweep: optionally interleave 4 nts of phase C per
                    # cc (used on ft0 to fill the DMA ramp).
                    units = [ring.unit()[0] for _ in range(QC)]
                    for cc in range(CC):
                        for qc in range(QC):
                            nc.tensor.matmul(
                                units[qc][:],
                                wqkF[ft][:, cc * 128:(cc + 1) * 128],
                                xT[cc][:, qc * 512:(qc + 1) * 512],
                                start=(cc == 0), stop=(cc == CC - 1))
                            if cc == CC - 1:
                                _evac_b(ft, qc, units[qc])
                        if c_sweep is not None:
                            _c_pass(c_sweep, cc)

                def _c_pass(g, cc):
                    # one contraction step of phase C for nts 4g..4g+3
                    for u in range(4):
                        nt = 4 * g + u
                        nc.tensor.matmul(
                            cunits[u][:],
                            xT[cc][:, nt * 128:(nt + 1) * 128], wv[cc],
                            start=(cc == 0), stop=(cc == CC - 1))
                        if cc == CC - 1:
                            _evac_c(nt, cunits[u])

                def _c_sweep(g):
                    for cc in range(CC):
                        _c_pass(g, cc)

                # ft0 + first C sweep interleave with the xT DMA stream;
                # the rest run dense from SBUF.
                _b_group(0, c_sweep=0)
                _b_group(1)
                _c_sweep(1)
                _b_group(2)
                _c_sweep(2)
                _b_group(3)
                _c_sweep(3)
                _b_group(4)
                _b_group(5)

            # ---------------- Phases D (attention) + E (proj) ----------------
            # qc-major so the output projection for a q-chunk can overlap the
            # next chunk's attention. Inner loop is pipelined per kt-triplet:
            # ring slots of 2 units per kt (pos 0-1 / 2-3 / 4-5); exp for kts
            # 0,1 of a triplet is one ScalarE ACTIVATE over ring[0:2048], kt 2
            # goes to VectorE via a Schraudolph fast-exp (int16 bit trick ->
            # bf16), offloading 1/3 of the exp work so ScalarE never gates the
            # PE. PV matmuls trail by one triplet.
            with ExitStack() as s23:
                ctxT_pool = s23.enter_context(tc.tile_pool(name="ctxT", bufs=3))
                ctxT = [ctxT_pool.tile([128, N], BF16, tag="ctxT",
                                       name=f"ctxT{i}") for i in range(3)]

                with ExitStack() as s2, ExitStack() as s3:
                    ctx_pool = s2.enter_context(
                        tc.tile_pool(name="ctxps", bufs=4, space="PSUM"))
                    exp1_pool = s2.enter_context(tc.tile_pool(name="et1", bufs=10))
                    small_pool = s2.enter_context(tc.tile_pool(name="small", bufs=6))
                    tmp_pool = s2.enter_context(tc.tile_pool(name="ctmp", bufs=2))
                    out_pool = s3.enter_context(tc.tile_pool(name="outT", bufs=4))

                    def _emit_proj(qcp, pair=None):
                        # E: out^T = W_proj^T ctx^T (+bias) for q chunk qcp;
                        # emitted 2 of-units at a time (pair != None) so the
                        # evac work spreads across kts instead of clumping
                        ofs = (range(C // 128) if pair is None
                               else range(2 * pair, 2 * pair + 2))
                        for of in ofs:
                            ps, _ = ring.unit()
                            for c2 in range(3):
                                nc.tensor.matmul(
                                    ps[:], wp[c2][:, of * 128:(of + 1) * 128],
                                    ctxT[c2][:, qcp * 512:(qcp + 1) * 512],
                                    start=(c2 == 0), stop=(c2 == 2))
                            ot = out_pool.tile([128, 512], BF16, tag="outT",
                                               name="ot")
                            if of % 2 == 0:
                                nc.vector.tensor_scalar_add(ot[:], ps[:],
                                                            b_p[:, of:of + 1])
                            else:
                                nc.scalar.activation(ot[:], ps[:], AF.Identity,
                                                     bias=b_p[:, of:of + 1])
                            nc.sync.dma_start(
                                out_d[of * 128:(of + 1) * 128,
                                      qcp * 512:(qcp + 1) * 512], ot[:])

                    pv_defer = []   # deferred PV work, carried ACROSS groups
                    # The softmax-normalize chain is staged ONE OP PER KT
                    # into the next group's loop (rsB@7, recip@9/10,
                    # norm@12/14) so it never clumps ahead of an exp in
                    # either engine FIFO; the PE meanwhile chews the
                    # carried PV backlog, so the ring pipeline never
                    # drains between head-pair groups.
                    pending_rs = []
                    pending_recip = []
                    pending_norm = []

                    def _emit_pv(batch):
                        for ctxps, et_ap, kk, lh in batch["work"]:
                            nc.tensor.matmul(
                                ctxps[0:65, :],
                                vaug[kk][:, lh * 65:lh * 65 + 65],
                                et_ap,
                                start=(kk == 0), stop=(kk == KT - 1))
                        if batch["evac"] is not None:
                            batch["evac"]()

                    def _make_evac(hp, qc, ctxps, last=False):
                        def _evac():
                            # rowsum rows PSUM -> SBUF (the custom-DVE recip
                            # below is SBUF-only)
                            rsA = small_pool.tile([1, 512], F32, tag="rsA")
                            nc.scalar.activation(rsA[:], ctxps[0][64:65, :],
                                                 AF.Identity)

                            def _rsB():
                                rsB = small_pool.tile([1, 512], F32, tag="rsB")
                                nc.scalar.activation(rsB[:],
                                                     ctxps[1][64:65, :],
                                                     AF.Identity)
                                pending_recip.append(
                                    lambda: _emit_recip(1, rsB))
                            pending_rs.append(_rsB)
                            pending_recip.append(lambda: _emit_recip(0, rsA))

                        def _emit_recip(i, rs):
                            recip = small_pool.tile([1, 512], F32,
                                                    tag=f"recip{i}")
                            nc.vector.reciprocal_approx_fast(recip[:], rs[:])
                            bc = small_pool.tile([64, 512], F32, tag=f"bc{i}")
                            nc.gpsimd.partition_broadcast(bc[:], recip[:])
                            pending_norm.append(lambda: _emit_norm(i, bc))

                        def _emit_norm(i, bc):
                            if i == 0:
                                nc.vector.scalar_tensor_tensor(
                                    ctxT[hp][0:64, qc * 512:(qc + 1) * 512],
                                    ctxps[0][0:64, :], 1.0, bc[:],
                                    op0=ALU.mult, op1=ALU.mult)
                                return
                            ctmp = tmp_pool.tile([64, 512], BF16, tag="ctmp",
                                                 name="ctmp")
                            nc.vector.scalar_tensor_tensor(
                                ctmp[:], ctxps[1][0:64, :], 1.0, bc[:],
                                op0=ALU.mult, op1=ALU.mult)
                            dq = nc.scalar if last else nc.sync
                            dq.dma_start(
                                ctxT[hp][64:128, qc * 512:(qc + 1) * 512],
                                ctmp[:])
                            dq.dma_start(
                                aot_d[hp * 128:(hp + 1) * 128,
                                      qc * 512:(qc + 1) * 512],
                                ctxT[hp][:, qc * 512:(qc + 1) * 512])
                        return _evac

                    # exp engine per kt alternates strictly (even -> ScalarE
                    # ACT, odd -> DVE Schraudolph fast-exp): with only 2
                    # ring slots, ST(kt) waits on exp(kt-2), which is
                    # always the OTHER engine — consecutive exps never
                    # queue behind each other on one engine.
                    EXP_DVE = {1, 3, 5, 7, 9, 11, 13, 15}

                    for qc in range(QC):
                        for hp in range(3):
                            ctxps = [ctx_pool.tile([128, 512], F32, tag="ctxps",
                                                   name=f"ctxps{i}")
                                     for i in range(2)]
                            last_grp = (qc == QC - 1 and hp == 2)
                            for kt in range(KT):
                                # flush deferred PV before the ST that may
                                # wait on a ring slot, so the PE has work
                                # queued ahead of the wait; the last group
                                # drains its backlog early to shrink the
                                # kernel tail
                                thresh = max(2, 6 - kt) if last_grp else 6
                                while len(pv_defer) > thresh:
                                    _emit_pv(pv_defer.pop(0))
                                if hp == 1 and qc > 0 and kt in (2, 3, 4):
                                    _emit_proj(qc - 1, kt - 2)
                                r = None
                                for ab in range(2):
                                    sts, pos = ring.unit()
                                    if r is None:
                                        r = (pos % 6) // 2
                                    ho = ab * 64
                                    nc.tensor.matmul(
                                        sts,
                                        qkT[3 + hp][ho:ho + 64, kt * 128:(kt + 1) * 128],
                                        qkT[hp][ho:ho + 64, qc * 512:(qc + 1) * 512],
                                        start=True, stop=True,
                                        tile_position=(ho, 0))
                                et1 = exp1_pool.tile([128, 1024], BF16,
                                                     tag="et1", name="et1")
                                if last_grp and kt >= 12:
                                    # halve exp latency at the kernel tail:
                                    # each engine takes one 512 half
                                    nc.vector.tensor_scalar(
                                        et1[:].bitcast(I16)[:, 0:512],
                                        ring.slot(r)[:, 0:512],
                                        FE_A, FE_B,
                                        op0=ALU.mult, op1=ALU.add)
                                    nc.scalar.activation(
                                        et1[:, 512:1024],
                                        ring.slot(r)[:, 512:1024], AF.Exp)
                                elif kt in EXP_DVE:
                                    nc.vector.tensor_scalar(
                                        et1[:].bitcast(I16),
                                        ring.slot(r)[:],
                                        FE_A, FE_B,
                                        op0=ALU.mult, op1=ALU.add)
                                else:
                                    nc.scalar.activation(
                                        et1[:], ring.slot(r)[:], AF.Exp)
                                if kt == 7 and pending_rs:
                                    pending_rs.pop(0)()
                                elif kt in (9, 10) and pending_recip:
                                    pending_recip.pop(0)()
                                elif kt in (12, 14) and pending_norm:
                                    pending_norm.pop(0)()
                                batch = {
                                    "work": [
                                        (ctxps[0], et1[:, 0:512], kt, hp * 2),
                                        (ctxps[1], et1[:, 512:1024], kt,
                                         hp * 2 + 1)],
                                    "evac": None}
                                if kt == KT - 1:
                                    batch["evac"] = _make_evac(
                                        hp, qc, ctxps, last=last_grp)
                                pv_defer.append(batch)
                    # final drain + staged last projection: 4 of-units'
                    # c2 0/1 partial products (the full 4-unit ring) run
                    # while the last normalize chain completes; only the
                    # c2=2 matmuls wait on the final ctxT.
                    while pv_defer:
                        _emit_pv(pv_defer.pop(0))
                    while pending_rs:
                        pending_rs.pop(0)()
                    while pending_recip:
                        pending_recip.pop(0)()
                    qf = (QC - 1) * 512
                    last_ps = []
                    for of in range(4):
                        ps, _ = ring.unit()
                        last_ps.append(ps)
                        for c2 in range(2):
                            nc.tensor.matmul(
                                ps[:], wp[c2][:, of * 128:(of + 1) * 128],
                                ctxT[c2][:, qf:qf + 512],
                                start=(c2 == 0), stop=False)
                    while pending_norm:
                        pending_norm.pop(0)()

                    def _finish_of(of, ps, c2_range):
                        for c2 in c2_range:
                            nc.tensor.matmul(
                                ps[:], wp[c2][:, of * 128:(of + 1) * 128],
                                ctxT[c2][:, qf:qf + 512],
                                start=(c2 == 0), stop=(c2 == 2))
                        ot = out_pool.tile([128, 512], BF16, tag="outT",
                                           name="ot")
                        if of % 2 == 0:
                            nc.vector.tensor_scalar_add(ot[:], ps[:],
                                                        b_p[:, of:of + 1])
                        else:
                            nc.scalar.activation(ot[:], ps[:], AF.Identity,
                                                 bias=b_p[:, of:of + 1])
                        nc.sync.dma_start(
                            out_d[of * 128:(of + 1) * 128, qf:qf + 512],
                            ot[:])

                    for of in range(4):
                        _finish_of(of, last_ps[of], [2])
                    for of in range(4, 6):
                        ps, _ = ring.unit()
                        _finish_of(of, ps, [0, 1, 2])


    nc.compile()
    return nc


def _get_nc(repeat=1):
    key = ("nc", repeat)
    if key not in _CACHE:
        _CACHE[key] = _build_nc(repeat)
    return _CACHE[key]


def _prep_inputs(x, W_qkv, b_qkv, W_proj, b_proj):
    x = np.ascontiguousarray(np.asarray(x, dtype=np.float32))
    W_qkv = np.asarray(W_qkv, dtype=np.float32)
    b_qkv = np.asarray(b_qkv, dtype=np.float32)
    W_proj = np.asarray(W_proj, dtype=np.float32)
    b_proj = np.asarray(b_proj, dtype=np.float32)

    bf = ml_dtypes.bfloat16
    in_maps = []
    for c in range(N_CORES):
        b, g = divmod(c, G)
        sl = slice(g * CL, (g + 1) * CL)
        w_q = W_qkv[:, 0:C][:, sl] * SCALE
        w_k = W_qkv[:, C:2 * C][:, sl]
        w_v = np.ascontiguousarray(W_qkv[:, 2 * C:3 * C][:, sl])
        b_q = b_qkv[0:C][sl] * SCALE
        b_k = b_qkv[C:2 * C][sl]
        b_v = b_qkv[2 * C:3 * C][sl]
        w_qk = np.concatenate([w_q, w_k], axis=1)
        # device layout: [p, (ft, cc, 128)] so each wqkF slab DMA reads
        # 1.5KB contiguous per partition
        w_qkF = np.ascontiguousarray(
            w_qk.reshape(CC, 128, FQK, 128).transpose(1, 2, 0, 3)
            .reshape(128, FQK * CC * 128))
        b_qk = np.ascontiguousarray(
            np.concatenate([b_q, b_k]).reshape(FQK, 128).T)
        w_p = np.ascontiguousarray(W_proj[sl, :])
        bp = b_proj if g == 0 else np.zeros_like(b_proj)
        b_p = np.ascontiguousarray(bp.reshape(C // 128, 128).T)
        in_maps.append({
            "xT": np.ascontiguousarray(x[b].T).astype(bf),
            "w_qk": w_qkF.astype(bf),
            "w_v": np.ascontiguousarray(
                w_v.reshape(CC, 128, CL).transpose(1, 0, 2)
                .reshape(128, CC * CL)).astype(bf),
            "w_p": np.ascontiguousarray(
                w_p.reshape(3, 128, C).transpose(1, 0, 2)
                .reshape(128, 3 * C)).astype(bf),
            "b_qk": b_qk,
            "b_v": np.ascontiguousarray(b_v[None, :]).astype(np.float32),
            "b_p": b_p,
        })
    return in_maps


def run_cores(in_maps, **kw):
    nc = _get_nc()
    return run_bass_kernel_spmd(nc, in_maps, list(range(N_CORES)), **kw)


def gather(results):
    out = np.empty((B, N, C), dtype=np.float32)
    attn_out = np.empty((B, N, C), dtype=np.float32)
    for b in range(B):
        r0 = results[b * G + 0]
        r1 = results[b * G + 1]
        attn_out[b, :, 0:CL] = r0["attn_out_t"].T
        attn_out[b, :, CL:C] = r1["attn_out_t"].T
        out[b] = r0["out_t"].T.astype(np.float32)
        out[b] += r1["out_t"].T.astype(np.float32)
    return out, attn_out


def kernel(x, W_qkv, b_qkv, W_proj, b_proj):
    in_maps = _prep_inputs(x, W_qkv, b_qkv, W_proj, b_proj)
    res = run_cores(in_maps)
    return gather(res.results)



# revision 41
# speedup vs baseline: 1.2396x; 1.2396x over previous
"""Fused multi-head attention + output projection for Trainium2 (Bass/Tile).

Problem: B=4, N=2048, C=768, H=12 heads x D=64.
  qkv = x @ W_qkv + b_qkv ; q,k,v per head ; attn = softmax(q k^T / sqrt(D))
  attn_out = (attn @ v) merged ; out = attn_out @ W_proj + b_proj
  returns (out, attn_out)

Sharding over 8 NeuronCores: core c = (b, g) with b = batch (4), g = head
group (2 groups of 6 heads).  Data-parallel over batch, tensor-parallel over
heads: W_qkv columns / W_proj rows are split per group; the N x N attention
matrix stays core-local.  Host only slices inputs and, on gather, transposes
the (feature-major) outputs and sums the two W_proj partial products per
batch.

Per-core device algorithm (all layouts feature-major "T" = [features, n]):
  xT = transpose(x_b)                       (PE transposes via identity)
  qkT[f, n] = W_qk^T x (+bias, q pre-scaled on host)      fp32r matmuls
  v[n, f] (+bias via ones-row matmul), stored bf16 augmented with a ones
      column per head -> PV matmul also yields softmax row-sums.
  Per head: S^T[k, q] = kT^T qT (no max subtraction needed: |S| <= ~6),
      P^T = exp(S^T) on ScalarE straight out of PSUM (bf16),
      ctx^T[d, q] (+rowsum row) = [v|1]^T @ P^T, normalize by 1/rowsum.
  out^T = W_proj^T ctx^T (+b_proj on group-0 cores only, via zeroed input).

All phases share one 6-bank PSUM "ring" of [128,512] units (plus 2 banks of
PV accumulators), so no PSUM pool boundary serializes phase transitions.
"""

import os
import numpy as np
import ml_dtypes
from contextlib import ExitStack

import concourse.bass as bass
import concourse.tile as tile
import concourse.mybir as mybir
from concourse import bacc
import concourse.bass_utils as _bass_utils
from concourse.bass_utils import run_bass_kernel_spmd

# walrus is invoked with --enable-ldw-opt=false by default, which forces a
# serial LDWEIGHTS before every MATMUL (~250us of PE time for this kernel).
_orig_run_command = _bass_utils.run_command


def _run_command_ldw(argv, **kw):
    argv = ["--enable-ldw-opt=true" if a == "--enable-ldw-opt=false" else a
            for a in argv]
    return _orig_run_command(argv, **kw)


# NOTE: tried --enable-ldw-opt=true: walrus rejects it for fp32/fp32r
# weights ("InstLdweights is not compatible with LDW optimization").
ENABLE_LDW_OPT = bool(os.environ.get("K_LDW_OPT"))
if ENABLE_LDW_OPT and _bass_utils.run_command is _orig_run_command:
    _bass_utils.run_command = _run_command_ldw

N_CORES = 8
B, N, C = 4, 2048, 768
H, D = 12, 64
G = 2                # head groups (tensor-parallel)
HL = H // G          # heads per core
CL = HL * D          # local feature width (384)
SCALE = D ** -0.5
NT = N // 128        # 16 row tiles
CC = C // 128        # 6 contraction chunks
QC = N // 512        # 4 q chunks of 512
KT = N // 128        # 16 k tiles
FQK = 2 * CL // 128  # 6 feature tiles for q|k

F32 = mybir.dt.float32
F32R = mybir.dt.float32r
BF16 = mybir.dt.bfloat16
I16 = mybir.dt.int16

# Schraudolph fast-exp in bf16 bit space: bf16_bits(exp(x)) ~ x*FE_A + FE_B.
# FE_B calibrated on HW (trunc semantics) for zero-mean relative error.
FE_A = 2.0 ** 7 / float(np.log(2.0))
FE_B = 16256.5 - 7.88
AF = mybir.ActivationFunctionType
ALU = mybir.AluOpType

_CACHE = {}


class Ring:
    """Rotating [128, 512] PSUM units across persistent 2-bank slot
    tiles. Separate tiles keep the tile-level dependency tracking
    per-slot: the ST matmul reusing slot s waits only on that slot's
    exp reader n_slots kts back, not on every in-flight ring access."""

    def __init__(self, slots):
        self.slots = slots          # tiles of [128, 1024]
        self.n = 2 * len(slots)
        self.pos = 0

    def unit(self, width=512):
        p = self.pos % self.n
        self.pos += 1
        return self.slots[p // 2][:, (p % 2) * 512:(p % 2) * 512 + width], p

    def slot_unit(self):
        p = self.pos % self.n
        assert p % 2 == 0
        self.pos += 2
        return self.slots[p // 2][:, :]

    def slot(self, r):
        return self.slots[r]


def _build_nc(repeat=1):
    nc = bacc.Bacc("TRN2", target_bir_lowering=False, debug=False,
                   num_devices=N_CORES)
    xt_d = nc.dram_tensor("xT", [C, N], BF16, kind="ExternalInput").ap()
    wqk_d = nc.dram_tensor("w_qk", [128, FQK * CC * 128], BF16,
                           kind="ExternalInput").ap()
    wv_d = nc.dram_tensor("w_v", [128, CC * CL], BF16, kind="ExternalInput").ap()
    wp_d = nc.dram_tensor("w_p", [128, 3 * C], BF16, kind="ExternalInput").ap()
    bqk_d = nc.dram_tensor("b_qk", [128, FQK], F32, kind="ExternalInput").ap()
    bv_d = nc.dram_tensor("b_v", [1, CL], F32, kind="ExternalInput").ap()
    bp_d = nc.dram_tensor("b_p", [128, C // 128], F32, kind="ExternalInput").ap()
    aot_d = nc.dram_tensor("attn_out_t", [CL, N], BF16, kind="ExternalOutput").ap()
    out_d = nc.dram_tensor("out_t", [C, N], BF16, kind="ExternalOutput").ap()
    DEBUG = bool(os.environ.get("K_DEBUG"))
    if DEBUG:
        dbg_rs = nc.dram_tensor("dbg_rs", [1, 512], F32, kind="ExternalOutput").ap()
        dbg_rA = nc.dram_tensor("dbg_rA", [1, 512], F32, kind="ExternalOutput").ap()
        dbg_bc = nc.dram_tensor("dbg_bc", [64, 512], F32, kind="ExternalOutput").ap()

    with tile.TileContext(nc) as tc:
      for _rep in range(repeat):
        with ExitStack() as top:
            const_pool = top.enter_context(tc.tile_pool(name="const", bufs=1))
            bias_pool = top.enter_context(tc.tile_pool(name="bias", bufs=3))
            qkT_pool = top.enter_context(tc.tile_pool(name="qkT", bufs=FQK))
            vaug_pool = top.enter_context(tc.tile_pool(name="vaug", bufs=NT))
            wp_pool = top.enter_context(tc.tile_pool(name="wp", bufs=1))
            ring_pool = top.enter_context(
                tc.tile_pool(name="ring", bufs=1, space="PSUM"))

            # 2 ring slots (4 banks) + 4 ctx accumulator banks = all 8
            # PSUM banks: the ctx double-buffering lets the PV backlog
            # carry across head-pair groups without draining the ring.
            ring = Ring([ring_pool.tile([128, 1024], F32, tag=f"ring{i}",
                                        name=f"ringt{i}") for i in range(2)])

            # x arrives pre-transposed from the host: plain parallel DMA
            # loads instead of the serialized transpose-xbar path.
            xT_pool_o = top.enter_context(tc.tile_pool(name="xT", bufs=CC))
            xT = [xT_pool_o.tile([128, N], BF16, tag="xT", name=f"xTt{i}")
                  for i in range(CC)]

            b_qk = bias_pool.tile([128, FQK], F32, tag="bqk")
            b_p = bias_pool.tile([128, C // 128], F32, tag="bp")
            b_v = bias_pool.tile([1, CL], F32, tag="bv")

            qkT = [qkT_pool.tile([128, N], BF16, tag="qkT", name=f"qkT{i}")
                   for i in range(FQK)]
            # per-head pitch 65: [v(64) | ones(1)] — the narrow stationary
            # keeps the PV ldweights at 65 columns (~54ns) instead of 128.
            vaug = [vaug_pool.tile([128, HL * 65], BF16, tag="vaug",
                                   name=f"vaug{i}") for i in range(NT)]
            # ------------- Phases A (xT), B (qkT), C (v) -------------
            with ExitStack() as s1:
                wv_pool = s1.enter_context(tc.tile_pool(name="wv", bufs=1))
                wv_all = wv_pool.tile([128, CC * CL], BF16, tag="wv",
                                      name="wv_all")
                wv = [wv_all[:, cc * CL:(cc + 1) * CL] for cc in range(CC)]

                wqk_pool = s1.enter_context(
                    tc.tile_pool(name="wqk", bufs=FQK))
                # ft-major weight slabs: one 0.2MB DMA unblocks a whole
                # B ft-group instead of needing all six row chunks.
                wqkF = [wqk_pool.tile([128, CC * 128], BF16, tag="wqk",
                                      name=f"wqkF{ft}") for ft in range(FQK)]
                cps_pool = s1.enter_context(
                    tc.tile_pool(name="cps", bufs=1, space="PSUM"))
                cunits = [cps_pool.tile([128, CL], F32, tag=f"cps{i}",
                                        name=f"cps{i}") for i in range(4)]

                def _dma_wqkF(ft):
                    nc.sync.dma_start(wqkF[ft][:],
                                      wqk_d[:, ft * 768:(ft + 1) * 768])

                # DMA issue order = need order. Weights ride the SP HWDGE
                # queue, xT the Act queue, so issue serialization (~0.7us
                # per dma_start) runs 2-wide. xT[0] is split so the very
                # first matmul starts after ~0.3 MB of traffic.
                _dma_wqkF(0)
                nc.scalar.dma_start(xT[0][:, 0:512], xt_d[0:128, 0:512])
                nc.sync.dma_start(wv_all[:], wv_d[:])
                nc.scalar.dma_start(xT[0][:, 512:N], xt_d[0:128, 512:N])
                for cc in range(1, CC):
                    nc.scalar.dma_start(xT[cc][:],
                                        xt_d[cc * 128:(cc + 1) * 128, :])
                nc.sync.dma_start(b_qk[:], bqk_d[:])
                nc.sync.dma_start(b_v[:], bv_d[:])
                for ft in range(1, FQK):
                    _dma_wqkF(ft)
                nc.sync.dma_start(b_p[:], bp_d[:])
                wp_all = wp_pool.tile([128, 3 * C], BF16, tag="wp",
                                      name="wp_all")
                nc.scalar.dma_start(wp_all[:], wp_d[:])
                wp = [wp_all[:, i * C:(i + 1) * C] for i in range(3)]

                # b_v broadcast once: the C evac folds the v-bias via
                # tensor_tensor, saving a 1-row matmul per nt tile.
                bvb = bias_pool.tile([128, CL], F32, tag="bvb")
                nc.gpsimd.partition_broadcast(bvb[:], b_v[:])

                def _evac_b(ft, qc, unit):
                    if qc % 2 == 0:
                        nc.vector.tensor_scalar_add(
                            qkT[ft][:, qc * 512:(qc + 1) * 512],
                            unit[:], b_qk[:, ft:ft + 1])
                    else:
                        nc.scalar.activation(
                            qkT[ft][:, qc * 512:(qc + 1) * 512], unit[:],
                            AF.Identity, bias=b_qk[:, ft:ft + 1])

                def _evac_c(nt, unit):
                    va3 = vaug[nt][:].rearrange("p (h e) -> p h e", e=65)
                    nc.vector.tensor_tensor(
                        va3[:, :, 0:64],
                        unit[:].rearrange("p (h e) -> p h e", e=64),
                        bvb[:].rearrange("p (h e) -> p h e", e=64),
                        op=ALU.add)
                    nc.vector.memset(va3[:, :, 64:65], 1.0)

                def _b_group(ft, c_sweep=None):
                    # B ft-group, cc-outer: the 4 q-chunk accumulators
                    # fill in DMA-arrival order so the PE streams each xT
                    # chunk as it lands; evacs ride inline with the last
                    # cc pass so the next group's units free up early.
                    # c_sweep: optionally interleave 4 nts of phase C per
                    # cc (used on ft0 to fill the DMA ramp).
                    units = [ring.unit()[0] for _ in range(QC)]
                    for cc in range(CC):
                        for qc in range(QC):
                            nc.tensor.matmul(
                                units[qc][:],
                                wqkF[ft][:, cc * 128:(cc + 1) * 128],
                                xT[cc][:, qc * 512:(qc + 1) * 512],
                                start=(cc == 0), stop=(cc == CC - 1))
                            if cc == CC - 1:
                                _evac_b(ft, qc, units[qc])
                        if c_sweep is not None:
                            _c_pass(c_sweep, cc)

                def _c_pass(g, cc):
                    # one contraction step of phase C for nts 4g..4g+3
                    for u in range(4):
                        nt = 4 * g + u
                        nc.tensor.matmul(
                            cunits[u][:],
                            xT[cc][:, nt * 128:(nt + 1) * 128], wv[cc],
                            start=(cc == 0), stop=(cc == CC - 1))
                        if cc == CC - 1:
                            _evac_c(nt, cunits[u])

                def _c_sweep(g):
                    for cc in range(CC):
                        _c_pass(g, cc)

                # ft0 + first C sweep interleave with the xT DMA stream;
                # the rest run dense from SBUF.
                _b_group(0, c_sweep=0)
                _b_group(1)
                _c_sweep(1)
                _b_group(2)
                _c_sweep(2)
                _b_group(3)
                _c_sweep(3)
                _b_group(4)
                _b_group(5)

            # ---------------- Phases D (attention) + E (proj) ----------------
            # qc-major so the output projection for a q-chunk can overlap the
            # next chunk's attention. Inner loop is pipelined per kt-triplet:
            # ring slots of 2 units per kt (pos 0-1 / 2-3 / 4-5); exp for kts
            # 0,1 of a triplet is one ScalarE ACTIVATE over ring[0:2048], kt 2
            # goes to VectorE via a Schraudolph fast-exp (int16 bit trick ->
            # bf16), offloading 1/3 of the exp work so ScalarE never gates the
            # PE. PV matmuls trail by one triplet.
            with ExitStack() as s23:
                ctxT_pool = s23.enter_context(tc.tile_pool(name="ctxT", bufs=3))
                ctxT = [ctxT_pool.tile([128, N], BF16, tag="ctxT",
                                       name=f"ctxT{i}") for i in range(3)]

                with ExitStack() as s2, ExitStack() as s3:
                    ctx_pool = s2.enter_context(
                        tc.tile_pool(name="ctxps", bufs=4, space="PSUM"))
                    exp1_pool = s2.enter_context(tc.tile_pool(name="et1", bufs=10))
                    small_pool = s2.enter_context(tc.tile_pool(name="small", bufs=6))
                    tmp_pool = s2.enter_context(tc.tile_pool(name="ctmp", bufs=2))
                    out_pool = s3.enter_context(tc.tile_pool(name="outT", bufs=4))

                    def _emit_proj(qcp, pair=None):
                        # E: out^T = W_proj^T ctx^T (+bias) for q chunk qcp;
                        # emitted 2 of-units at a time (pair != None) so the
                        # evac work spreads across kts instead of clumping
                        ofs = (range(C // 128) if pair is None
                               else range(2 * pair, 2 * pair + 2))
                        for of in ofs:
                            ps, _ = ring.unit()
                            for c2 in range(3):
                                nc.tensor.matmul(
                                    ps[:], wp[c2][:, of * 128:(of + 1) * 128],
                                    ctxT[c2][:, qcp * 512:(qcp + 1) * 512],
                                    start=(c2 == 0), stop=(c2 == 2))
                            ot = out_pool.tile([128, 512], BF16, tag="outT",
                                               name="ot")
                            if of % 2 == 0:
                                nc.vector.tensor_scalar_add(ot[:], ps[:],
                                                            b_p[:, of:of + 1])
                            else:
                                nc.scalar.activation(ot[:], ps[:], AF.Identity,
                                                     bias=b_p[:, of:of + 1])
                            nc.sync.dma_start(
                                out_d[of * 128:(of + 1) * 128,
                                      qcp * 512:(qcp + 1) * 512], ot[:])

                    pv_defer = []   # deferred PV work, carried ACROSS groups
                    # The softmax-normalize chain is staged ONE OP PER KT
                    # into the next group's loop (rsB@7, recip@9/10,
                    # norm@12/14) so it never clumps ahead of an exp in
                    # either engine FIFO; the PE meanwhile chews the
                    # carried PV backlog, so the ring pipeline never
                    # drains between head-pair groups.
                    pending_rs = []
                    pending_recip = []
                    pending_norm = []

                    def _emit_pv(batch):
                        for ctxps, et_ap, kk, lh in batch["work"]:
                            nc.tensor.matmul(
                                ctxps[0:65, :],
                                vaug[kk][:, lh * 65:lh * 65 + 65],
                                et_ap,
                                start=(kk == 0), stop=(kk == KT - 1))
                        if batch["evac"] is not None:
                            batch["evac"]()

                    def _make_evac(hp, qc, ctxps, last=False):
                        def _evac():
                            # rowsum rows PSUM -> SBUF (the custom-DVE recip
                            # below is SBUF-only)
                            rsA = small_pool.tile([1, 512], F32, tag="rsA")
                            nc.scalar.activation(rsA[:], ctxps[0][64:65, :],
                                                 AF.Identity)

                            def _rsB():
                                rsB = small_pool.tile([1, 512], F32, tag="rsB")
                                nc.scalar.activation(rsB[:],
                                                     ctxps[1][64:65, :],
                                                     AF.Identity)
                                pending_recip.append(
                                    lambda: _emit_recip(1, rsB))
                            pending_rs.append(_rsB)
                            pending_recip.append(lambda: _emit_recip(0, rsA))

                        def _emit_recip(i, rs):
                            recip = small_pool.tile([1, 512], F32,
                                                    tag=f"recip{i}")
                            nc.vector.reciprocal_approx_fast(recip[:], rs[:])
                            bc = small_pool.tile([64, 512], F32, tag=f"bc{i}")
                            nc.gpsimd.partition_broadcast(bc[:], recip[:])
                            pending_norm.append(lambda: _emit_norm(i, bc))

                        def _emit_norm(i, bc):
                            if i == 0:
                                nc.vector.scalar_tensor_tensor(
                                    ctxT[hp][0:64, qc * 512:(qc + 1) * 512],
                                    ctxps[0][0:64, :], 1.0, bc[:],
                                    op0=ALU.mult, op1=ALU.mult)
                                return
                            ctmp = tmp_pool.tile([64, 512], BF16, tag="ctmp",
                                                 name="ctmp")
                            nc.vector.scalar_tensor_tensor(
                                ctmp[:], ctxps[1][0:64, :], 1.0, bc[:],
                                op0=ALU.mult, op1=ALU.mult)
                            dq = nc.scalar if last else nc.sync
                            dq.dma_start(
                                ctxT[hp][64:128, qc * 512:(qc + 1) * 512],
                                ctmp[:])
                            dq.dma_start(
                                aot_d[hp * 128:(hp + 1) * 128,
                                      qc * 512:(qc + 1) * 512],
                                ctxT[hp][:, qc * 512:(qc + 1) * 512])
                        return _evac

                    # exp engine per kt alternates strictly (even -> ScalarE
                    # ACT, odd -> DVE Schraudolph fast-exp): with only 2
                    # ring slots, ST(kt) waits on exp(kt-2), which is
                    # always the OTHER engine — consecutive exps never
                    # queue behind each other on one engine.
                    EXP_DVE = {1, 3, 5, 7, 9, 11, 13, 15}

                    for qc in range(QC):
                        for hp in range(3):
                            ctxps = [ctx_pool.tile([128, 512], F32, tag="ctxps",
                                                   name=f"ctxps{i}")
                                     for i in range(2)]
                            last_grp = (qc == QC - 1 and hp == 2)
                            for kt in range(KT):
                                # flush deferred PV before the ST that may
                                # wait on a ring slot, so the PE has work
                                # queued ahead of the wait; the last group
                                # drains its backlog early to shrink the
                                # kernel tail
                                thresh = max(2, 6 - kt) if last_grp else 6
                                while len(pv_defer) > thresh:
                                    _emit_pv(pv_defer.pop(0))
                                if hp == 1 and qc > 0 and kt == 2:
                                    _emit_proj(qc - 1)
                                r = None
                                for ab in range(2):
                                    sts, pos = ring.unit()
                                    if r is None:
                                        r = (pos % 6) // 2
                                    ho = ab * 64
                                    nc.tensor.matmul(
                                        sts,
                                        qkT[3 + hp][ho:ho + 64, kt * 128:(kt + 1) * 128],
                                        qkT[hp][ho:ho + 64, qc * 512:(qc + 1) * 512],
                                        start=True, stop=True,
                                        tile_position=(ho, 0))
                                et1 = exp1_pool.tile([128, 1024], BF16,
                                                     tag="et1", name="et1")
                                if last_grp and kt >= 12:
                                    # halve exp latency at the kernel tail:
                                    # each engine takes one 512 half
                                    nc.vector.tensor_scalar(
                                        et1[:].bitcast(I16)[:, 0:512],
                                        ring.slot(r)[:, 0:512],
                                        FE_A, FE_B,
                                        op0=ALU.mult, op1=ALU.add)
                                    nc.scalar.activation(
                                        et1[:, 512:1024],
                                        ring.slot(r)[:, 512:1024], AF.Exp)
                                elif kt in EXP_DVE:
                                    nc.vector.tensor_scalar(
                                        et1[:].bitcast(I16),
                                        ring.slot(r)[:],
                                        FE_A, FE_B,
                                        op0=ALU.mult, op1=ALU.add)
                                else:
                                    nc.scalar.activation(
                                        et1[:], ring.slot(r)[:], AF.Exp)
                                if kt == 7 and pending_rs:
                                    pending_rs.pop(0)()
                                elif kt in (9, 10) and pending_recip:
                                    pending_recip.pop(0)()
                                elif kt in (12, 14) and pending_norm:
                                    pending_norm.pop(0)()
                                batch = {
                                    "work": [
                                        (ctxps[0], et1[:, 0:512], kt, hp * 2),
                                        (ctxps[1], et1[:, 512:1024], kt,
                                         hp * 2 + 1)],
                                    "evac": None}
                                if kt == KT - 1:
                                    batch["evac"] = _make_evac(
                                        hp, qc, ctxps, last=last_grp)
                                pv_defer.append(batch)
                    # final drain + staged last projection: 4 of-units'
                    # c2 0/1 partial products (the full 4-unit ring) run
                    # while the last normalize chain completes; only the
                    # c2=2 matmuls wait on the final ctxT.
                    while pv_defer:
                        _emit_pv(pv_defer.pop(0))
                    while pending_rs:
                        pending_rs.pop(0)()
                    while pending_recip:
                        pending_recip.pop(0)()
                    qf = (QC - 1) * 512
                    last_ps = []
                    for of in range(4):
                        ps, _ = ring.unit()
                        last_ps.append(ps)
                        for c2 in range(2):
                            nc.tensor.matmul(
                                ps[:], wp[c2][:, of * 128:(of + 1) * 128],
                                ctxT[c2][:, qf:qf + 512],
                                start=(c2 == 0), stop=False)
                    while pending_norm:
                        pending_norm.pop(0)()

                    def _finish_of(of, ps, c2_range):
                        for c2 in c2_range:
                            nc.tensor.matmul(
                                ps[:], wp[c2][:, of * 128:(of + 1) * 128],
                                ctxT[c2][:, qf:qf + 512],
                                start=(c2 == 0), stop=(c2 == 2))
                        ot = out_pool.tile([128, 512], BF16, tag="outT",
                                           name="ot")
                        if of % 2 == 0:
                            nc.vector.tensor_scalar_add(ot[:], ps[:],
                                                        b_p[:, of:of + 1])
                        else:
                            nc.scalar.activation(ot[:], ps[:], AF.Identity,
                                                 bias=b_p[:, of:of + 1])
                        nc.sync.dma_start(
                            out_d[of * 128:(of + 1) * 128, qf:qf + 512],
                            ot[:])

                    for of in range(4):
                        _finish_of(of, last_ps[of], [2])
                    for of in range(4, 6):
                        ps, _ = ring.unit()
                        _finish_of(of, ps, [0, 1, 2])


    nc.compile()
    return nc


def _get_nc(repeat=1):
    key = ("nc", repeat)
    if key not in _CACHE:
        _CACHE[key] = _build_nc(repeat)
    return _CACHE[key]


def _prep_inputs(x, W_qkv, b_qkv, W_proj, b_proj):
    x = np.ascontiguousarray(np.asarray(x, dtype=np.float32))
    W_qkv = np.asarray(W_qkv, dtype=np.float32)
    b_qkv = np.asarray(b_qkv, dtype=np.float32)
    W_proj = np.asarray(W_proj, dtype=np.float32)
    b_proj = np.asarray(b_proj, dtype=np.float32)

    bf = ml_dtypes.bfloat16
    in_maps = []
    for c in range(N_CORES):
        b, g = divmod(c, G)
        sl = slice(g * CL, (g + 1) * CL)
        w_q = W_qkv[:, 0:C][:, sl] * SCALE
        w_k = W_qkv[:, C:2 * C][:, sl]
        w_v = np.ascontiguousarray(W_qkv[:, 2 * C:3 * C][:, sl])
        b_q = b_qkv[0:C][sl] * SCALE
        b_k = b_qkv[C:2 * C][sl]
        b_v = b_qkv[2 * C:3 * C][sl]
        w_qk = np.concatenate([w_q, w_k], axis=1)
        # device layout: [p, (ft, cc, 128)] so each wqkF slab DMA reads
        # 1.5KB contiguous per partition
        w_qkF = np.ascontiguousarray(
            w_qk.reshape(CC, 128, FQK, 128).transpose(1, 2, 0, 3)
            .reshape(128, FQK * CC * 128))
        b_qk = np.ascontiguousarray(
            np.concatenate([b_q, b_k]).reshape(FQK, 128).T)
        w_p = np.ascontiguousarray(W_proj[sl, :])
        bp = b_proj if g == 0 else np.zeros_like(b_proj)
        b_p = np.ascontiguousarray(bp.reshape(C // 128, 128).T)
        in_maps.append({
            "xT": np.ascontiguousarray(x[b].T).astype(bf),
            "w_qk": w_qkF.astype(bf),
            "w_v": np.ascontiguousarray(
                w_v.reshape(CC, 128, CL).transpose(1, 0, 2)
                .reshape(128, CC * CL)).astype(bf),
            "w_p": np.ascontiguousarray(
                w_p.reshape(3, 128, C).transpose(1, 0, 2)
                .reshape(128, 3 * C)).astype(bf),
            "b_qk": b_qk,
            "b_v": np.ascontiguousarray(b_v[None, :]).astype(np.float32),
            "b_p": b_p,
        })
    return in_maps


def run_cores(in_maps, **kw):
    nc = _get_nc()
    return run_bass_kernel_spmd(nc, in_maps, list(range(N_CORES)), **kw)


def gather(results):
    out = np.empty((B, N, C), dtype=np.float32)
    attn_out = np.empty((B, N, C), dtype=np.float32)
    for b in range(B):
        r0 = results[b * G + 0]
        r1 = results[b * G + 1]
        attn_out[b, :, 0:CL] = r0["attn_out_t"].T
        attn_out[b, :, CL:C] = r1["attn_out_t"].T
        out[b] = r0["out_t"].T.astype(np.float32)
        out[b] += r1["out_t"].T.astype(np.float32)
    return out, attn_out


def kernel(x, W_qkv, b_qkv, W_proj, b_proj):
    in_maps = _prep_inputs(x, W_qkv, b_qkv, W_proj, b_proj)
    res = run_cores(in_maps)
    return gather(res.results)



# revision 42
# speedup vs baseline: 1.2519x; 1.0099x over previous
"""Fused multi-head attention + output projection for Trainium2 (Bass/Tile).

Problem: B=4, N=2048, C=768, H=12 heads x D=64.
  qkv = x @ W_qkv + b_qkv ; q,k,v per head ; attn = softmax(q k^T / sqrt(D))
  attn_out = (attn @ v) merged ; out = attn_out @ W_proj + b_proj
  returns (out, attn_out)

Sharding over 8 NeuronCores: core c = (b, g) with b = batch (4), g = head
group (2 groups of 6 heads).  Data-parallel over batch, tensor-parallel over
heads: W_qkv columns / W_proj rows are split per group; the N x N attention
matrix stays core-local.  Host only slices inputs and, on gather, transposes
the (feature-major) outputs and sums the two W_proj partial products per
batch.

Per-core device algorithm (all layouts feature-major "T" = [features, n]):
  xT = transpose(x_b)                       (PE transposes via identity)
  qkT[f, n] = W_qk^T x (+bias, q pre-scaled on host)      fp32r matmuls
  v[n, f] (+bias via ones-row matmul), stored bf16 augmented with a ones
      column per head -> PV matmul also yields softmax row-sums.
  Per head: S^T[k, q] = kT^T qT (no max subtraction needed: |S| <= ~6),
      P^T = exp(S^T) on ScalarE straight out of PSUM (bf16),
      ctx^T[d, q] (+rowsum row) = [v|1]^T @ P^T, normalize by 1/rowsum.
  out^T = W_proj^T ctx^T (+b_proj on group-0 cores only, via zeroed input).

All phases share one 6-bank PSUM "ring" of [128,512] units (plus 2 banks of
PV accumulators), so no PSUM pool boundary serializes phase transitions.
"""

import os
import numpy as np
import ml_dtypes
from contextlib import ExitStack

import concourse.bass as bass
import concourse.tile as tile
import concourse.mybir as mybir
from concourse import bacc
import concourse.bass_utils as _bass_utils
from concourse.bass_utils import run_bass_kernel_spmd

# walrus is invoked with --enable-ldw-opt=false by default, which forces a
# serial LDWEIGHTS before every MATMUL (~250us of PE time for this kernel).
_orig_run_command = _bass_utils.run_command


def _run_command_ldw(argv, **kw):
    argv = ["--enable-ldw-opt=true" if a == "--enable-ldw-opt=false" else a
            for a in argv]
    return _orig_run_command(argv, **kw)


# NOTE: tried --enable-ldw-opt=true: walrus rejects it for fp32/fp32r
# weights ("InstLdweights is not compatible with LDW optimization").
ENABLE_LDW_OPT = bool(os.environ.get("K_LDW_OPT"))
if ENABLE_LDW_OPT and _bass_utils.run_command is _orig_run_command:
    _bass_utils.run_command = _run_command_ldw

N_CORES = 8
B, N, C = 4, 2048, 768
H, D = 12, 64
G = 2                # head groups (tensor-parallel)
HL = H // G          # heads per core
CL = HL * D          # local feature width (384)
SCALE = D ** -0.5
NT = N // 128        # 16 row tiles
CC = C // 128        # 6 contraction chunks
QC = N // 512        # 4 q chunks of 512
KT = N // 128        # 16 k tiles
FQK = 2 * CL // 128  # 6 feature tiles for q|k

F32 = mybir.dt.float32
F32R = mybir.dt.float32r
BF16 = mybir.dt.bfloat16
I16 = mybir.dt.int16

# Schraudolph fast-exp in bf16 bit space: bf16_bits(exp(x)) ~ x*FE_A + FE_B.
# FE_B calibrated on HW (trunc semantics) for zero-mean relative error.
FE_A = 2.0 ** 7 / float(np.log(2.0))
FE_B = 16256.5 - 7.88
AF = mybir.ActivationFunctionType
ALU = mybir.AluOpType

_CACHE = {}


class Ring:
    """Rotating [128, 512] PSUM units across persistent 2-bank slot
    tiles. Separate tiles keep the tile-level dependency tracking
    per-slot: the ST matmul reusing slot s waits only on that slot's
    exp reader n_slots kts back, not on every in-flight ring access."""

    def __init__(self, slots):
        self.slots = slots          # tiles of [128, 1024]
        self.n = 2 * len(slots)
        self.pos = 0

    def unit(self, width=512):
        p = self.pos % self.n
        self.pos += 1
        return self.slots[p // 2][:, (p % 2) * 512:(p % 2) * 512 + width], p

    def slot_unit(self):
        p = self.pos % self.n
        assert p % 2 == 0
        self.pos += 2
        return self.slots[p // 2][:, :]

    def slot(self, r):
        return self.slots[r]


def _build_nc(repeat=1):
    nc = bacc.Bacc("TRN2", target_bir_lowering=False, debug=False,
                   num_devices=N_CORES)
    xt_d = nc.dram_tensor("xT", [C, N], BF16, kind="ExternalInput").ap()
    wqk_d = nc.dram_tensor("w_qk", [128, FQK * CC * 128], BF16,
                           kind="ExternalInput").ap()
    wv_d = nc.dram_tensor("w_v", [128, CC * CL], BF16, kind="ExternalInput").ap()
    wp_d = nc.dram_tensor("w_p", [128, 3 * C], BF16, kind="ExternalInput").ap()
    bqk_d = nc.dram_tensor("b_qk", [128, FQK], F32, kind="ExternalInput").ap()
    bv_d = nc.dram_tensor("b_v", [1, CL], F32, kind="ExternalInput").ap()
    bp_d = nc.dram_tensor("b_p", [128, C // 128], F32, kind="ExternalInput").ap()
    aot_d = nc.dram_tensor("attn_out_t", [CL, N], BF16, kind="ExternalOutput").ap()
    out_d = nc.dram_tensor("out_t", [C, N], BF16, kind="ExternalOutput").ap()
    DEBUG = bool(os.environ.get("K_DEBUG"))
    if DEBUG:
        dbg_rs = nc.dram_tensor("dbg_rs", [1, 512], F32, kind="ExternalOutput").ap()
        dbg_rA = nc.dram_tensor("dbg_rA", [1, 512], F32, kind="ExternalOutput").ap()
        dbg_bc = nc.dram_tensor("dbg_bc", [64, 512], F32, kind="ExternalOutput").ap()

    with tile.TileContext(nc) as tc:
      for _rep in range(repeat):
        with ExitStack() as top:
            const_pool = top.enter_context(tc.tile_pool(name="const", bufs=1))
            bias_pool = top.enter_context(tc.tile_pool(name="bias", bufs=3))
            qkT_pool = top.enter_context(tc.tile_pool(name="qkT", bufs=FQK))
            vaug_pool = top.enter_context(tc.tile_pool(name="vaug", bufs=NT))
            wp_pool = top.enter_context(tc.tile_pool(name="wp", bufs=1))
            ring_pool = top.enter_context(
                tc.tile_pool(name="ring", bufs=1, space="PSUM"))

            # 2 ring slots (4 banks) + 4 ctx accumulator banks = all 8
            # PSUM banks: the ctx double-buffering lets the PV backlog
            # carry across head-pair groups without draining the ring.
            ring = Ring([ring_pool.tile([128, 1024], F32, tag=f"ring{i}",
                                        name=f"ringt{i}") for i in range(2)])

            # x arrives pre-transposed from the host: plain parallel DMA
            # loads instead of the serialized transpose-xbar path.
            xT_pool_o = top.enter_context(tc.tile_pool(name="xT", bufs=CC))
            xT = [xT_pool_o.tile([128, N], BF16, tag="xT", name=f"xTt{i}")
                  for i in range(CC)]

            b_qk = bias_pool.tile([128, FQK], F32, tag="bqk")
            b_p = bias_pool.tile([128, C // 128], F32, tag="bp")
            b_v = bias_pool.tile([1, CL], F32, tag="bv")

            qkT = [qkT_pool.tile([128, N], BF16, tag="qkT", name=f"qkT{i}")
                   for i in range(FQK)]
            # per-head pitch 65: [v(64) | ones(1)] — the narrow stationary
            # keeps the PV ldweights at 65 columns (~54ns) instead of 128.
            vaug = [vaug_pool.tile([128, HL * 65], BF16, tag="vaug",
                                   name=f"vaug{i}") for i in range(NT)]
            # ------------- Phases A (xT), B (qkT), C (v) -------------
            with ExitStack() as s1:
                wv_pool = s1.enter_context(tc.tile_pool(name="wv", bufs=1))
                wv_all = wv_pool.tile([128, CC * CL], BF16, tag="wv",
                                      name="wv_all")
                wv = [wv_all[:, cc * CL:(cc + 1) * CL] for cc in range(CC)]

                wqk_pool = s1.enter_context(
                    tc.tile_pool(name="wqk", bufs=FQK))
                # ft-major weight slabs: one 0.2MB DMA unblocks a whole
                # B ft-group instead of needing all six row chunks.
                wqkF = [wqk_pool.tile([128, CC * 128], BF16, tag="wqk",
                                      name=f"wqkF{ft}") for ft in range(FQK)]
                cps_pool = s1.enter_context(
                    tc.tile_pool(name="cps", bufs=1, space="PSUM"))
                cunits = [cps_pool.tile([128, CL], F32, tag=f"cps{i}",
                                        name=f"cps{i}") for i in range(4)]

                def _dma_wqkF(ft):
                    nc.sync.dma_start(wqkF[ft][:],
                                      wqk_d[:, ft * 768:(ft + 1) * 768])

                # DMA issue order = need order. Weights ride the SP HWDGE
                # queue, xT the Act queue, so issue serialization (~0.7us
                # per dma_start) runs 2-wide. xT[0] is split so the very
                # first matmul starts after ~0.3 MB of traffic.
                _dma_wqkF(0)
                nc.scalar.dma_start(xT[0][:, 0:512], xt_d[0:128, 0:512])
                nc.sync.dma_start(wv_all[:], wv_d[:])
                nc.scalar.dma_start(xT[0][:, 512:N], xt_d[0:128, 512:N])
                for cc in range(1, CC):
                    nc.scalar.dma_start(xT[cc][:],
                                        xt_d[cc * 128:(cc + 1) * 128, :])
                nc.sync.dma_start(b_qk[:], bqk_d[:])
                nc.sync.dma_start(b_v[:], bv_d[:])
                for ft in range(1, FQK):
                    _dma_wqkF(ft)
                nc.sync.dma_start(b_p[:], bp_d[:])
                wp_all = wp_pool.tile([128, 3 * C], BF16, tag="wp",
                                      name="wp_all")
                nc.scalar.dma_start(wp_all[:], wp_d[:])
                wp = [wp_all[:, i * C:(i + 1) * C] for i in range(3)]

                # b_v broadcast once: the C evac folds the v-bias via
                # tensor_tensor, saving a 1-row matmul per nt tile.
                bvb = bias_pool.tile([128, CL], F32, tag="bvb")
                nc.gpsimd.partition_broadcast(bvb[:], b_v[:])

                def _evac_b(ft, qc, unit):
                    if qc % 2 == 0:
                        nc.vector.tensor_scalar_add(
                            qkT[ft][:, qc * 512:(qc + 1) * 512],
                            unit[:], b_qk[:, ft:ft + 1])
                    else:
                        nc.scalar.activation(
                            qkT[ft][:, qc * 512:(qc + 1) * 512], unit[:],
                            AF.Identity, bias=b_qk[:, ft:ft + 1])

                def _evac_c(nt, unit):
                    va3 = vaug[nt][:].rearrange("p (h e) -> p h e", e=65)
                    nc.vector.tensor_tensor(
                        va3[:, :, 0:64],
                        unit[:].rearrange("p (h e) -> p h e", e=64),
                        bvb[:].rearrange("p (h e) -> p h e", e=64),
                        op=ALU.add)
                    nc.vector.memset(va3[:, :, 64:65], 1.0)

                def _b_group(ft, c_sweep=None):
                    # B ft-group, cc-outer: the 4 q-chunk accumulators
                    # fill in DMA-arrival order so the PE streams each xT
                    # chunk as it lands; evacs ride inline with the last
                    # cc pass so the next group's units free up early.
                    # c_sweep: optionally interleave 4 nts of phase C per
                    # cc (used on ft0 to fill the DMA ramp).
                    units = [ring.unit()[0] for _ in range(QC)]
                    for cc in range(CC):
                        for qc in range(QC):
                            nc.tensor.matmul(
                                units[qc][:],
                                wqkF[ft][:, cc * 128:(cc + 1) * 128],
                                xT[cc][:, qc * 512:(qc + 1) * 512],
                                start=(cc == 0), stop=(cc == CC - 1))
                            if cc == CC - 1:
                                _evac_b(ft, qc, units[qc])
                        if c_sweep is not None:
                            _c_pass(c_sweep, cc)

                def _c_pass(g, cc):
                    # one contraction step of phase C for nts 4g..4g+3
                    for u in range(4):
                        nt = 4 * g + u
                        nc.tensor.matmul(
                            cunits[u][:],
                            xT[cc][:, nt * 128:(nt + 1) * 128], wv[cc],
                            start=(cc == 0), stop=(cc == CC - 1))
                        if cc == CC - 1:
                            _evac_c(nt, cunits[u])

                def _c_sweep(g):
                    for cc in range(CC):
                        _c_pass(g, cc)

                # ft0 + first C sweep interleave with the xT DMA stream;
                # the rest run dense from SBUF.
                _b_group(0, c_sweep=0)
                _b_group(1)
                _c_sweep(1)
                _b_group(2)
                _c_sweep(2)
                _b_group(3)
                _c_sweep(3)
                _b_group(4)
                _b_group(5)

            # ---------------- Phases D (attention) + E (proj) ----------------
            # qc-major so the output projection for a q-chunk can overlap the
            # next chunk's attention. Inner loop is pipelined per kt-triplet:
            # ring slots of 2 units per kt (pos 0-1 / 2-3 / 4-5); exp for kts
            # 0,1 of a triplet is one ScalarE ACTIVATE over ring[0:2048], kt 2
            # goes to VectorE via a Schraudolph fast-exp (int16 bit trick ->
            # bf16), offloading 1/3 of the exp work so ScalarE never gates the
            # PE. PV matmuls trail by one triplet.
            with ExitStack() as s23:
                ctxT_pool = s23.enter_context(tc.tile_pool(name="ctxT", bufs=3))
                ctxT = [ctxT_pool.tile([128, N], BF16, tag="ctxT",
                                       name=f"ctxT{i}") for i in range(3)]

                with ExitStack() as s2, ExitStack() as s3:
                    ctx_pool = s2.enter_context(
                        tc.tile_pool(name="ctxps", bufs=4, space="PSUM"))
                    exp1_pool = s2.enter_context(tc.tile_pool(name="et1", bufs=10))
                    small_pool = s2.enter_context(tc.tile_pool(name="small", bufs=6))
                    tmp_pool = s2.enter_context(tc.tile_pool(name="ctmp", bufs=2))
                    out_pool = s3.enter_context(tc.tile_pool(name="outT", bufs=4))

                    def _emit_proj(qcp, pair=None):
                        # E: out^T = W_proj^T ctx^T (+bias) for q chunk qcp;
                        # emitted 2 of-units at a time (pair != None) so the
                        # evac work spreads across kts instead of clumping
                        ofs = (range(C // 128) if pair is None
                               else range(2 * pair, 2 * pair + 2))
                        for of in ofs:
                            ps, _ = ring.unit()
                            for c2 in range(3):
                                nc.tensor.matmul(
                                    ps[:], wp[c2][:, of * 128:(of + 1) * 128],
                                    ctxT[c2][:, qcp * 512:(qcp + 1) * 512],
                                    start=(c2 == 0), stop=(c2 == 2))
                            ot = out_pool.tile([128, 512], BF16, tag="outT",
                                               name="ot")
                            if of % 2 == 0:
                                nc.vector.tensor_scalar_add(ot[:], ps[:],
                                                            b_p[:, of:of + 1])
                            else:
                                nc.scalar.activation(ot[:], ps[:], AF.Identity,
                                                     bias=b_p[:, of:of + 1])
                            nc.sync.dma_start(
                                out_d[of * 128:(of + 1) * 128,
                                      qcp * 512:(qcp + 1) * 512], ot[:])

                    pv_defer = []   # deferred PV work, carried ACROSS groups
                    # The softmax-normalize chain is staged ONE OP PER KT
                    # into the next group's loop (rsB@7, recip@9/10,
                    # norm@12/14) so it never clumps ahead of an exp in
                    # either engine FIFO; the PE meanwhile chews the
                    # carried PV backlog, so the ring pipeline never
                    # drains between head-pair groups.
                    pending_rs = []
                    pending_recip = []
                    pending_norm = []

                    def _emit_pv(batch):
                        for ctxps, et_ap, kk, lh in batch["work"]:
                            nc.tensor.matmul(
                                ctxps[0:65, :],
                                vaug[kk][:, lh * 65:lh * 65 + 65],
                                et_ap,
                                start=(kk == 0), stop=(kk == KT - 1))
                        if batch["evac"] is not None:
                            batch["evac"]()

                    def _make_evac(hp, qc, ctxps, last=False):
                        def _evac():
                            # rowsum rows PSUM -> SBUF (the custom-DVE recip
                            # below is SBUF-only)
                            rsA = small_pool.tile([1, 512], F32, tag="rsA")
                            nc.scalar.activation(rsA[:], ctxps[0][64:65, :],
                                                 AF.Identity)

                            def _rsB():
                                rsB = small_pool.tile([1, 512], F32, tag="rsB")
                                nc.scalar.activation(rsB[:],
                                                     ctxps[1][64:65, :],
                                                     AF.Identity)
                                pending_recip.append(
                                    lambda: _emit_recip(1, rsB))
                            pending_rs.append(_rsB)
                            pending_recip.append(lambda: _emit_recip(0, rsA))

                        def _emit_recip(i, rs):
                            recip = small_pool.tile([1, 512], F32,
                                                    tag=f"recip{i}")
                            nc.vector.reciprocal_approx_fast(recip[:], rs[:])
                            bc = small_pool.tile([64, 512], F32, tag=f"bc{i}")
                            nc.gpsimd.partition_broadcast(bc[:], recip[:])
                            pending_norm.append(lambda: _emit_norm(i, bc))

                        def _emit_norm(i, bc):
                            if i == 0:
                                nc.vector.scalar_tensor_tensor(
                                    ctxT[hp][0:64, qc * 512:(qc + 1) * 512],
                                    ctxps[0][0:64, :], 1.0, bc[:],
                                    op0=ALU.mult, op1=ALU.mult)
                                return
                            ctmp = tmp_pool.tile([64, 512], BF16, tag="ctmp",
                                                 name="ctmp")
                            nc.vector.scalar_tensor_tensor(
                                ctmp[:], ctxps[1][0:64, :], 1.0, bc[:],
                                op0=ALU.mult, op1=ALU.mult)
                            dq = nc.scalar if last else nc.sync
                            dq.dma_start(
                                ctxT[hp][64:128, qc * 512:(qc + 1) * 512],
                                ctmp[:])
                            dq.dma_start(
                                aot_d[hp * 128:(hp + 1) * 128,
                                      qc * 512:(qc + 1) * 512],
                                ctxT[hp][:, qc * 512:(qc + 1) * 512])
                        return _evac

                    # exp engine per kt alternates strictly (even -> ScalarE
                    # ACT, odd -> DVE Schraudolph fast-exp): with only 2
                    # ring slots, ST(kt) waits on exp(kt-2), which is
                    # always the OTHER engine — consecutive exps never
                    # queue behind each other on one engine.
                    EXP_DVE = {1, 3, 5, 7, 9, 11, 13, 15}

                    for qc in range(QC):
                        for hp in range(3):
                            ctxps = [ctx_pool.tile([128, 512], F32, tag="ctxps",
                                                   name=f"ctxps{i}")
                                     for i in range(2)]
                            last_grp = (qc == QC - 1 and hp == 2)
                            for kt in range(KT):
                                # flush deferred PV before the ST that may
                                # wait on a ring slot, so the PE has work
                                # queued ahead of the wait; the last group
                                # drains its backlog early to shrink the
                                # kernel tail
                                thresh = max(2, 6 - kt) if last_grp else 6
                                while len(pv_defer) > thresh:
                                    _emit_pv(pv_defer.pop(0))
                                if hp == 1 and qc > 0 and kt == 2:
                                    _emit_proj(qc - 1)
                                r = None
                                for ab in range(2):
                                    sts, pos = ring.unit()
                                    if r is None:
                                        r = (pos % 6) // 2
                                    ho = ab * 64
                                    nc.tensor.matmul(
                                        sts,
                                        qkT[3 + hp][ho:ho + 64, kt * 128:(kt + 1) * 128],
                                        qkT[hp][ho:ho + 64, qc * 512:(qc + 1) * 512],
                                        start=True, stop=True,
                                        tile_position=(ho, 0))
                                et1 = exp1_pool.tile([128, 1024], BF16,
                                                     tag="et1", name="et1")
                                if last_grp and kt >= 12:
                                    # halve exp latency at the kernel tail:
                                    # each engine takes one 512 half
                                    nc.vector.tensor_scalar(
                                        et1[:].bitcast(I16)[:, 0:512],
                                        ring.slot(r)[:, 0:512],
                                        FE_A, FE_B,
                                        op0=ALU.mult, op1=ALU.add)
                                    nc.scalar.activation(
                                        et1[:, 512:1024],
                                        ring.slot(r)[:, 512:1024], AF.Exp)
                                elif kt in EXP_DVE:
                                    nc.vector.tensor_scalar(
                                        et1[:].bitcast(I16),
                                        ring.slot(r)[:],
                                        FE_A, FE_B,
                                        op0=ALU.mult, op1=ALU.add)
                                else:
                                    nc.scalar.activation(
                                        et1[:], ring.slot(r)[:], AF.Exp)
                                if kt == 7 and pending_rs:
                                    pending_rs.pop(0)()
                                elif kt in (9, 10) and pending_recip:
                                    pending_recip.pop(0)()
                                elif kt in (12, 14) and pending_norm:
                                    pending_norm.pop(0)()
                                batch = {
                                    "work": [
                                        (ctxps[0], et1[:, 0:512], kt, hp * 2),
                                        (ctxps[1], et1[:, 512:1024], kt,
                                         hp * 2 + 1)],
                                    "evac": None}
                                if kt == KT - 1:
                                    batch["evac"] = _make_evac(
                                        hp, qc, ctxps, last=last_grp)
                                pv_defer.append(batch)
                    # final drain + staged last projection: 4 of-units'
                    # c2 0/1 partial products (the full 4-unit ring) run
                    # while the last normalize chain completes; only the
                    # c2=2 matmuls wait on the final ctxT.
                    while pv_defer:
                        _emit_pv(pv_defer.pop(0))
                    while pending_rs:
                        pending_rs.pop(0)()
                    while pending_recip:
                        pending_recip.pop(0)()
                    qf = (QC - 1) * 512
                    last_ps = []
                    for of in range(6):
                        if of < 4:
                            ps, _ = ring.unit()
                        else:
                            # the previous head-pair's ctx banks are free
                            # by now — borrow them for the last two units
                            ps = ctx_pool.tile([128, 512], F32, tag="ctxps",
                                               name=f"ctxps{of - 4}")
                        last_ps.append(ps)
                        for c2 in range(2):
                            nc.tensor.matmul(
                                ps[:], wp[c2][:, of * 128:(of + 1) * 128],
                                ctxT[c2][:, qf:qf + 512],
                                start=(c2 == 0), stop=False)
                    while pending_norm:
                        pending_norm.pop(0)()
                    for of in range(6):
                        ps = last_ps[of]
                        nc.tensor.matmul(
                            ps[:], wp[2][:, of * 128:(of + 1) * 128],
                            ctxT[2][:, qf:qf + 512],
                            start=False, stop=True)
                        ot = out_pool.tile([128, 512], BF16, tag="outT",
                                           name="ot")
                        if of % 2 == 0:
                            nc.vector.tensor_scalar_add(ot[:], ps[:],
                                                        b_p[:, of:of + 1])
                        else:
                            nc.scalar.activation(ot[:], ps[:], AF.Identity,
                                                 bias=b_p[:, of:of + 1])
                        nc.sync.dma_start(
                            out_d[of * 128:(of + 1) * 128, qf:qf + 512],
                            ot[:])


    nc.compile()
    return nc


def _get_nc(repeat=1):
    key = ("nc", repeat)
    if key not in _CACHE:
        _CACHE[key] = _build_nc(repeat)
    return _CACHE[key]


def _prep_inputs(x, W_qkv, b_qkv, W_proj, b_proj):
    x = np.ascontiguousarray(np.asarray(x, dtype=np.float32))
    W_qkv = np.asarray(W_qkv, dtype=np.float32)
    b_qkv = np.asarray(b_qkv, dtype=np.float32)
    W_proj = np.asarray(W_proj, dtype=np.float32)
    b_proj = np.asarray(b_proj, dtype=np.float32)

    bf = ml_dtypes.bfloat16
    in_maps = []
    for c in range(N_CORES):
        b, g = divmod(c, G)
        sl = slice(g * CL, (g + 1) * CL)
        w_q = W_qkv[:, 0:C][:, sl] * SCALE
        w_k = W_qkv[:, C:2 * C][:, sl]
        w_v = np.ascontiguousarray(W_qkv[:, 2 * C:3 * C][:, sl])
        b_q = b_qkv[0:C][sl] * SCALE
        b_k = b_qkv[C:2 * C][sl]
        b_v = b_qkv[2 * C:3 * C][sl]
        w_qk = np.concatenate([w_q, w_k], axis=1)
        # device layout: [p, (ft, cc, 128)] so each wqkF slab DMA reads
        # 1.5KB contiguous per partition
        w_qkF = np.ascontiguousarray(
            w_qk.reshape(CC, 128, FQK, 128).transpose(1, 2, 0, 3)
            .reshape(128, FQK * CC * 128))
        b_qk = np.ascontiguousarray(
            np.concatenate([b_q, b_k]).reshape(FQK, 128).T)
        w_p = np.ascontiguousarray(W_proj[sl, :])
        bp = b_proj if g == 0 else np.zeros_like(b_proj)
        b_p = np.ascontiguousarray(bp.reshape(C // 128, 128).T)
        in_maps.append({
            "xT": np.ascontiguousarray(x[b].T).astype(bf),
            "w_qk": w_qkF.astype(bf),
            "w_v": np.ascontiguousarray(
                w_v.reshape(CC, 128, CL).transpose(1, 0, 2)
                .reshape(128, CC * CL)).astype(bf),
            "w_p": np.ascontiguousarray(
                w_p.reshape(3, 128, C).transpose(1, 0, 2)
                .reshape(128, 3 * C)).astype(bf),
            "b_qk": b_qk,
            "b_v": np.ascontiguousarray(b_v[None, :]).astype(np.float32),
            "b_p": b_p,
        })
    return in_maps


def run_cores(in_maps, **kw):
    nc = _get_nc()
    return run_bass_kernel_spmd(nc, in_maps, list(range(N_CORES)), **kw)


def gather(results):
    out = np.empty((B, N, C), dtype=np.float32)
    attn_out = np.empty((B, N, C), dtype=np.float32)
    for b in range(B):
        r0 = results[b * G + 0]
        r1 = results[b * G + 1]
        attn_out[b, :, 0:CL] = r0["attn_out_t"].T
        attn_out[b, :, CL:C] = r1["attn_out_t"].T
        out[b] = r0["out_t"].T.astype(np.float32)
        out[b] += r1["out_t"].T.astype(np.float32)
    return out, attn_out


def kernel(x, W_qkv, b_qkv, W_proj, b_proj):
    in_maps = _prep_inputs(x, W_qkv, b_qkv, W_proj, b_proj)
    res = run_cores(in_maps)
    return gather(res.results)

